# revision 5
# baseline (speedup 1.0000x reference)
"""CLIPAttention (B=16,T=577,E=1024,H=16,D=64 + 77 textual kv) on 8 trn2 cores.

Sharding: data-parallel over batch (2 batches/core), weights replicated.
All matmuls bf16 with fp32 PSUM accumulation.

Layout plan per core (b in {0,1} local batches):
  hsT   [k,t]   host-pretransposed bf16
  QT,KT [e,t]   proj with M=e (bias = per-partition scalar, scale folded in Wq)
  V     [t,e']  proj with M=t, e' = head-spread cols (h*65+d, col h*65+64 = ones)
                bias bv + ones via K=1 ones-row matmul
  attnT [s,t]   QK^T with K=64, two heads row-packed in the PE array
  expT  [s,t]   exp on ScalarE, bf16
  AV    M=65    lhsT = v_aug[s, 65] -> psum [65, t]: rows 0..63 out_unT, row 64 = Z
  norm          rz = recip(Z) -> gpsimd partition_broadcast -> DVE mult -> aoT [e,t]
  out   [t,e]   M=t out-proj, bias via ones-row matmul
  k/v hidden rows: PE-transpose KT -> [t,e] f32, V psum -> f32 stage -> DMA
  k/v textual rows: assembled on host (w * textual_kv)
"""

import sys

for _p in ("/opt/trn_rl_repo",):
    if _p not in sys.path:
        sys.path.insert(0, _p)

import numpy as np
import ml_dtypes

BF16 = ml_dtypes.bfloat16

B, T, E, H, D = 16, 577, 1024, 16, 64
STEX, S = 77, 654
NCORES, BLOC = 8, 2
SCALE = float(D) ** -0.5

T_CHUNKS = [(0, 128), (128, 128), (256, 128), (384, 128), (512, 65)]
NCH = [(0, 512), (512, 65)]  # psum column chunks for 577-wide tiles
# (t_chunk_index or -1 for textual, start within group, size)
S_CHUNKS = [(-1, 0, STEX)] + [(i, t0, tsz) for i, (t0, tsz) in enumerate(T_CHUNKS)]

_CACHE = {}


def _build_nc():
    import concourse.tile as tile
    import concourse.mybir as mybir
    from concourse import bacc
    from concourse.masks import make_identity
    from contextlib import ExitStack

    f32 = mybir.dt.float32
    bf = mybir.dt.bfloat16
    Exp = mybir.ActivationFunctionType.Exp

    nc = bacc.Bacc("TRN2", target_bir_lowering=False, debug=False)

    hsT_d = nc.dram_tensor("hsT", [BLOC, E, T], bf, kind="ExternalInput").ap()
    wq_d = nc.dram_tensor("wqT", [E, E], bf, kind="ExternalInput").ap()
    wk_d = nc.dram_tensor("wkT", [E, E], bf, kind="ExternalInput").ap()
    wv_d = nc.dram_tensor("wvT", [E, H * 65], bf, kind="ExternalInput").ap()
    wo_d = nc.dram_tensor("woT", [E, E], bf, kind="ExternalInput").ap()
    bqr_d = nc.dram_tensor("bqr", [1, E], bf, kind="ExternalInput").ap()
    bkr_d = nc.dram_tensor("bkr", [1, E], bf, kind="ExternalInput").ap()
    bvr_d = nc.dram_tensor("bvr", [1, H * 65], bf, kind="ExternalInput").ap()
    bor_d = nc.dram_tensor("bor", [1, E], bf, kind="ExternalInput").ap()
    ktex_d = nc.dram_tensor("ktexT", [BLOC, 128, 8 * STEX], bf, kind="ExternalInput").ap()
    vtex_d = nc.dram_tensor("vtex", [BLOC, STEX, H * 65], bf, kind="ExternalInput").ap()

    out_d = nc.dram_tensor("out", [BLOC, T, E], f32, kind="ExternalOutput").ap()
    k_d = nc.dram_tensor("k_hid", [BLOC * H, T, D], f32, kind="ExternalOutput").ap()
    v_d = nc.dram_tensor("v_hid", [BLOC * H, T, D], f32, kind="ExternalOutput").ap()

    with tile.TileContext(nc) as tc, ExitStack() as ctx:
        consts = ctx.enter_context(tc.tile_pool(name="consts", bufs=1))
        wts = ctx.enter_context(tc.tile_pool(name="wts", bufs=1))
        hp = ctx.enter_context(tc.tile_pool(name="hp", bufs=2))
        qkv = ctx.enter_context(tc.tile_pool(name="qkv", bufs=1))
        expp = ctx.enter_context(tc.tile_pool(name="expp", bufs=18))
        stage = ctx.enter_context(tc.tile_pool(name="stage", bufs=4))
        smalls = ctx.enter_context(tc.tile_pool(name="smalls", bufs=3))
        pps = ctx.enter_context(tc.tile_pool(name="pps", bufs=4, space="PSUM"))

        identity = consts.tile([128, 128], bf)
        make_identity(nc, identity)
        ones_row = consts.tile([1, 512], bf)
        nc.vector.memset(ones_row, 1.0)

        wq = wts.tile([128, 8, E], bf)
        nc.sync.dma_start(wq, wq_d.rearrange("(j p) e -> p j e", p=128))
        wk = wts.tile([128, 8, E], bf)
        nc.sync.dma_start(wk, wk_d.rearrange("(j p) e -> p j e", p=128))
        wv = wts.tile([128, 8, H * 65], bf)
        nc.sync.dma_start(wv, wv_d.rearrange("(j p) e -> p j e", p=128))
        wo = wts.tile([128, 8, E], bf)
        nc.sync.dma_start(wo, wo_d.rearrange("(j p) e -> p j e", p=128))
        bqr = consts.tile([1, E], bf)
        nc.sync.dma_start(bqr, bqr_d)
        bkr = consts.tile([1, E], bf)
        nc.sync.dma_start(bkr, bkr_d)
        bvr = consts.tile([1, H * 65], bf)
        nc.sync.dma_start(bvr, bvr_d)
        bor = consts.tile([1, E], bf)
        nc.sync.dma_start(bor, bor_d)

        for b in range(BLOC):
            hst = hp.tile([128, 8, T], bf, tag="hst")
            nc.sync.dma_start(hst, hsT_d[b].rearrange("(j p) t -> p j t", p=128))
            ktex = hp.tile([128, 8 * STEX], bf, tag="ktex")
            nc.sync.dma_start(ktex, ktex_d[b])
            vtex = hp.tile([STEX, H * 65], bf, tag="vtex")
            nc.sync.dma_start(vtex, vtex_d[b])

            # ---- Q and K projections: psum[e_tile, t] = sum_k W^T[k, e] * hsT[k, t]
            qt = qkv.tile([128, 8, T], bf, tag="qt")
            kt = qkv.tile([128, 8, T], bf, tag="kt")
            for dst, w, bias in ((qt, wq, bqr), (kt, wk, bkr)):
                for m in range(8):
                    ps = pps.tile([128, T], f32, tag="ps")
                    for kk in range(8):
                        for c0, cw in NCH:
                            nc.tensor.matmul(
                                ps[:, c0 : c0 + cw],
                                w[:, kk, m * 128 : (m + 1) * 128],
                                hst[:, kk, c0 : c0 + cw],
                                start=(kk == 0),
                                stop=False,
                            )
                    for c0, cw in NCH:
                        nc.tensor.matmul(
                            ps[:, c0 : c0 + cw],
                            bias[0:1, m * 128 : (m + 1) * 128],
                            ones_row[0:1, 0:cw],
                            start=False,
                            stop=True,
                        )
                    nc.vector.tensor_copy(dst[:, m, :], ps[:, :])

            # ---- K hidden output: PE-transpose KT -> [t, e] f32 -> DMA
            for tt, (t0, tsz) in enumerate(T_CHUNKS):
                kstage = stage.tile([128, E], f32, tag="stage")
                for half in range(2):
                    pst = pps.tile([128, 512], bf, tag="ps")
                    for mm in range(4):
                        m = half * 4 + mm
                        nc.tensor.transpose(
                            pst[0:tsz, mm * 128 : (mm + 1) * 128],
                            kt[:, m, t0 : t0 + tsz],
                            identity,
                        )
                    nc.vector.tensor_copy(
                        kstage[0:tsz, half * 512 : (half + 1) * 512], pst[0:tsz, :]
                    )
                nc.sync.dma_start(
                    k_d[b * H : (b + 1) * H, t0 : t0 + tsz, :].rearrange("h t d -> t h d"),
                    kstage[0:tsz, :].rearrange("t (h d) -> t h d", d=D),
                )

            # ---- V projection: psum[t, h*65+d] (+ bias row incl. ones col)
            v_sb = qkv.tile([128, 5, H * 65], bf, tag="v_sb")
            for tt, (t0, tsz) in enumerate(T_CHUNKS):
                vstage = stage.tile([128, E], f32, tag="stage")
                for half in range(2):
                    base = half * 520
                    ps = pps.tile([128, 520], f32, tag="ps")
                    for kk in range(8):
                        for c0, cw in ((0, 512), (512, 8)):
                            nc.tensor.matmul(
                                ps[0:tsz, c0 : c0 + cw],
                                hst[:, kk, t0 : t0 + tsz],
                                wv[:, kk, base + c0 : base + c0 + cw],
                                start=(kk == 0),
                                stop=False,
                            )
                    for c0, cw in ((0, 512), (512, 8)):
                        nc.tensor.matmul(
                            ps[0:tsz, c0 : c0 + cw],
                            ones_row[0:1, 0:tsz],
                            bvr[0:1, base + c0 : base + c0 + cw],
                            start=False,
                            stop=True,
                        )
                    nc.scalar.copy(v_sb[0:tsz, tt, base : base + 520], ps[0:tsz, :])
                    nc.vector.tensor_copy(
                        vstage[0:tsz, half * 512 : (half + 1) * 512].rearrange(
                            "t (h d) -> t h d", d=D
                        ),
                        ps[0:tsz, :].rearrange("t (h x) -> t h x", x=65)[:, :, 0:D],
                    )
                nc.sync.dma_start(
                    v_d[b * H : (b + 1) * H, t0 : t0 + tsz, :].rearrange("h t d -> t h d"),
                    vstage[0:tsz, :].rearrange("t (h d) -> t h d", d=D),
                )

            # ---- Attention, head pairs (2p, 2p+1) row-packed in the PE array
            aoT = qkv.tile([128, 8, T], bf, tag="aoT")
            for pair in range(8):
                expts = ([], [])
                for tt, s0, ssz in S_CHUNKS:
                    for hh in range(2):
                        pb = hh * 64
                        ps = pps.tile([128, T], f32, tag="ps")
                        if tt < 0:
                            lhsT = ktex[pb : pb + 64, pair * STEX : (pair + 1) * STEX]
                        else:
                            lhsT = kt[pb : pb + 64, pair, s0 : s0 + ssz]
                        for c0, cw in NCH:
                            nc.tensor.matmul(
                                ps[0:ssz, c0 : c0 + cw],
                                lhsT,
                                qt[pb : pb + 64, pair, c0 : c0 + cw],
                                start=True,
                                stop=True,
                            )
                        et = expp.tile([128, T], bf, tag="expt")
                        nc.scalar.activation(et[0:ssz, :], ps[0:ssz, :], Exp)
                        expts[hh].append(et)

                for hh in range(2):
                    h = 2 * pair + hh
                    pb = hh * 64
                    pav = pps.tile([65, T], f32, tag="ps")
                    for sc, (tt, s0, ssz) in enumerate(S_CHUNKS):
                        if tt < 0:
                            lhsT = vtex[0:STEX, h * 65 : (h + 1) * 65]
                        else:
                            lhsT = v_sb[0:ssz, tt, h * 65 : (h + 1) * 65]
                        et = expts[hh][sc]
                        for c0, cw in NCH:
                            nc.tensor.matmul(
                                pav[0:65, c0 : c0 + cw],
                                lhsT,
                                et[0:ssz, c0 : c0 + cw],
                                start=(sc == 0),
                                stop=(sc == 5),
                            )
                    zrow = smalls.tile([1, T], f32, tag="zrow")
                    nc.scalar.copy(zrow, pav[64:65, :])
                    rz = smalls.tile([1, T], f32, tag="rz")
                    nc.vector.reciprocal_approx_fast(rz, zrow)
                    rzb = smalls.tile([64, T], f32, tag="rzb")
                    nc.gpsimd.partition_broadcast(rzb, rz)
                    nc.vector.tensor_mul(aoT[pb : pb + 64, pair, :], pav[0:64, :], rzb)

            # ---- Out projection: psum[t, e] = sum_e' aoT[e', t] * WoT[e', e] + bo
            for tt, (t0, tsz) in enumerate(T_CHUNKS):
                ostage = stage.tile([128, E], f32, tag="stage")
                for half in range(2):
                    pso = pps.tile([128, 512], f32, tag="ps")
                    c0 = half * 512
                    for kk in range(8):
                        nc.tensor.matmul(
                            pso[0:tsz, :],
                            aoT[:, kk, t0 : t0 + tsz],
                            wo[:, kk, c0 : c0 + 512],
                            start=(kk == 0),
                            stop=False,
                        )
                    nc.tensor.matmul(
                        pso[0:tsz, :],
                        ones_row[0:1, 0:tsz],
                        bor[0:1, c0 : c0 + 512],
                        start=False,
                        stop=True,
                    )
                    nc.vector.tensor_copy(ostage[0:tsz, c0 : c0 + 512], pso[0:tsz, :])
                nc.sync.dma_start(out_d[b, t0 : t0 + tsz, :], ostage[0:tsz, :])

    nc.compile()
    return nc


def _get_nc():
    if "nc" not in _CACHE:
        _CACHE["nc"] = _build_nc()
    return _CACHE["nc"]


def _prep_in_maps(hidden_states, textual_kv, Wq, bq, Wk, bk, Wv, bv, Wo, bo, tkv_weight):
    hs = np.asarray(hidden_states, np.float32)
    tkv = np.asarray(textual_kv, np.float32)
    Wq = np.asarray(Wq, np.float32)
    Wk = np.asarray(Wk, np.float32)
    Wv = np.asarray(Wv, np.float32)
    Wo = np.asarray(Wo, np.float32)
    bq = np.asarray(bq, np.float32)
    bk = np.asarray(bk, np.float32)
    bv = np.asarray(bv, np.float32)
    bo = np.asarray(bo, np.float32)
    w = float(np.asarray(tkv_weight, np.float32).reshape(-1)[0])

    hsT = np.ascontiguousarray(hs.transpose(0, 2, 1)).astype(BF16)  # [B, E, T]
    wqT = np.ascontiguousarray(Wq.T * SCALE).astype(BF16)
    wkT = np.ascontiguousarray(Wk.T).astype(BF16)
    woT = np.ascontiguousarray(Wo.T).astype(BF16)
    wvT = np.zeros((E, H * 65), np.float32)
    wvT.reshape(E, H, 65)[:, :, 0:D] = Wv.T.reshape(E, H, D)
    wvT = wvT.astype(BF16)
    bqr = (bq * SCALE).reshape(1, E).astype(BF16)
    bkr = bk.reshape(1, E).astype(BF16)
    bvr = np.zeros((1, H * 65), np.float32)
    bvr.reshape(H, 65)[:, 0:D] = bv.reshape(H, D)
    bvr.reshape(H, 65)[:, D] = 1.0
    bvr = bvr.astype(BF16)
    bor = bo.reshape(1, E).astype(BF16)

    tk0 = (w * tkv[0]).astype(np.float32)  # [B, H, 77, D]
    tv0 = (w * tkv[1]).astype(np.float32)
    # ktexT[b]: [hh*64+d, pair*77+s]
    ktexT = (
        tk0.reshape(B, 8, 2, STEX, D).transpose(0, 2, 4, 1, 3).reshape(B, 128, 8 * STEX)
    ).astype(BF16)
    vtex = np.zeros((B, STEX, H, 65), np.float32)
    vtex[:, :, :, 0:D] = tv0.transpose(0, 2, 1, 3)
    vtex[:, :, :, D] = 1.0
    vtex = vtex.reshape(B, STEX, H * 65).astype(BF16)

    in_maps = []
    for c in range(NCORES):
        in_maps.append(
            {
                "hsT": np.ascontiguousarray(hsT[c * BLOC : (c + 1) * BLOC]),
                "wqT": wqT,
                "wkT": wkT,
                "wvT": wvT,
                "woT": woT,
                "bqr": bqr,
                "bkr": bkr,
                "bvr": bvr,
                "bor": bor,
                "ktexT": np.ascontiguousarray(ktexT[c * BLOC : (c + 1) * BLOC]),
                "vtex": np.ascontiguousarray(vtex[c * BLOC : (c + 1) * BLOC]),
            }
        )
    return in_maps, tk0, tv0


def _assemble(results, tk0, tv0):
    out = np.empty((B, T, E), np.float32)
    k_full = np.empty((B * H, S, D), np.float32)
    v_full = np.empty((B * H, S, D), np.float32)
    k_full[:, 0:STEX, :] = tk0.reshape(B * H, STEX, D)
    v_full[:, 0:STEX, :] = tv0.reshape(B * H, STEX, D)
    for c in range(NCORES):
        r = results[c]
        out[c * BLOC : (c + 1) * BLOC] = r["out"]
        k_full[c * BLOC * H : (c + 1) * BLOC * H, STEX:, :] = r["k_hid"]
        v_full[c * BLOC * H : (c + 1) * BLOC * H, STEX:, :] = r["v_hid"]
    return out, k_full, v_full


def _run_cached(in_maps):
    """Run the prebuilt module on 8 cores, memoizing the jitted executable."""
    if "runner" not in _CACHE:
        from concourse import bass_utils

        def runner(maps):
            return bass_utils.run_bass_kernel_spmd(
                _get_nc(), maps, core_ids=list(range(NCORES))
            ).results

        _CACHE["runner"] = runner
    return _CACHE["runner"](in_maps)


def kernel(hidden_states, textual_kv, Wq, bq, Wk, bk, Wv, bv, Wo, bo, tkv_weight):
    in_maps, tk0, tv0 = _prep_in_maps(
        hidden_states, textual_kv, Wq, bq, Wk, bk, Wv, bv, Wo, bo, tkv_weight
    )
    results = _run_cached(in_maps)
    return _assemble(results, tk0, tv0)
